# revision 21
# baseline (speedup 1.0000x reference)
"""Multi-head attention layer (L=2048, B=2, D=1024, H=16) on 8 Trainium2 cores.

Sharding: batch*heads across cores — core c handles batch c//4, heads
4*(c%4)..4*(c%4)+4.  Tensor-parallel W_in column slice (per-head) and W_out
row slice; per-core partial outputs are summed on the host (2 groups of 4).

Device program (identical SPMD program, per-core data), organized as a
single PE conveyor:
  - 7 input DMAs: packed weights wT=[Wqk|Wv] (2), xT in m-quarters (4),
    woT (1).  All fp16.  Fewer DMAs = less serialized HWDGE generation
    (625ns each) and the m-quarter order lets attention start as soon as
    the first half of the tokens has landed.
  - projections are emitted as fine chunks (512-token psum tiles for q/k,
    128-token for v) and spliced just-in-time into the attention blocks'
    PE instruction stream wherever the exp pipeline would otherwise be
    the pacer; only the minimal prefix (q pair0 m0:1024, k pair0 m0:1024,
    v mc0..3) runs before attention block 0.
  - attention inner loop is software-pipelined: AV(mc-1) is emitted after
    S(mc), so the exp (ACT engine, ~1?s/tile) never stalls the PE.
  - AV uses interleaved ones-columns in v_sb so one matmul produces both
    z^T and the softmax row sums (cost model charges by output columns,
    so the extra 64 M-rows are free).
  - out-proj psum tiles ride the S-tag rotation; their copies run on the
    (otherwise idle) Pool engine and the output DMAs are issued from the
    ACT queue to keep SP/HWDGE free for input streaming.
"""

import sys

for _p in ("/opt/trn_rl_repo",):
    if _p not in sys.path:
        sys.path.append(_p)

import numpy as np

L, B, D, H = 2048, 2, 1024, 16
HD = 64
NCORES = 8
HPC = 4              # heads per core
J = HPC * HD         # 256 per-core head-dim slice
KC = D // 128        # 8 contraction chunks
P = 128

_COMPILED = None
_MARKS = []


def _build():
    import concourse.bacc as bacc
    import concourse.mybir as mybir
    import concourse.tile as tile
    from contextlib import ExitStack

    f32 = mybir.dt.float32
    f32r = mybir.dt.float32r
    f16 = mybir.dt.float16
    Exp = mybir.ActivationFunctionType.Exp
    Mult = mybir.AluOpType.mult

    del _MARKS[:]
    nc = bacc.Bacc("TRN2", target_bir_lowering=False, debug=False)

    xT_d = nc.dram_tensor("xT", (D, L), f16, kind="ExternalInput")
    w_d = nc.dram_tensor("wT", (D, 3 * J), f16, kind="ExternalInput")
    wo_d = nc.dram_tensor("woT", (J, D), f16, kind="ExternalInput")
    out_d = nc.dram_tensor("out_p", (L, D), f16, kind="ExternalOutput")

    with tile.TileContext(nc) as tc, ExitStack() as ctx:
        pers = ctx.enter_context(tc.tile_pool(name="pers", bufs=1))
        psum = ctx.enter_context(tc.tile_pool(name="psum", bufs=2, space="PSUM"))
        att = ctx.enter_context(tc.tile_pool(name="att", bufs=3))

        qk_sb = pers.tile([P, 4, L], f16)           # chunks 0,1: q^T; 2,3: k^T
        v_sb = pers.tile([P, 16, HPC, P], f32r)     # ones cols 0:64, v 64:128
        zn_sb = pers.tile([P, 2, L], f16)           # normalized z^T
        wo_sb = pers.tile([P, 2, D], f16)
        xT_sb = pers.tile([P, KC, L], f16)
        w_sb = pers.tile([P, KC, 3 * J], f16)   # [jc0 | jc2 | Wv | jc1 | jc3]
        WCOL = {0: 0, 2: 128, 1: 512, 3: 640}   # qk jc -> column offset
        WV0 = 256                               # Wv columns 256:512

        out_ap = out_d.ap().rearrange("(t p) o -> p t o", p=P)
        xT_ap = xT_d.ap().rearrange("(kc p) m -> p kc m", p=P)
        w_ap = w_d.ap().rearrange("(kc p) j -> p kc j", p=P)
        wo_ap = wo_d.ap().rearrange("(dc p) o -> p dc o", p=P)

        # input DMAs: weight chunks interleaved with x quarters so the
        # first projection chains can start as early as possible (chains
        # run kc 0..7, so w kc0:4 + xQ0 unblocks the first 4 matmuls)
        nc.sync.dma_start(w_sb[:, :, 0:512], w_ap[:, :, 0:512])
        nc.sync.dma_start(xT_sb[:, :, 0:512], xT_ap[:, :, 0:512])
        nc.sync.dma_start(xT_sb[:, :, 512:1024], xT_ap[:, :, 512:1024])
        nc.sync.dma_start(w_sb[:, :, 512:768], w_ap[:, :, 512:768])
        for q4 in range(2, 4):
            sl = slice(q4 * 512, (q4 + 1) * 512)
            nc.sync.dma_start(xT_sb[:, :, sl], xT_ap[:, :, sl])
        nc.sync.dma_start(wo_sb[:], wo_ap[:])

        # ones columns at 0:64 for every head — keeps the softmax sums on
        # psum partitions 0-63 where the custom-DVE reciprocal works (it
        # silently corrupts at base partition 64).  memset on an f32r
        # tile fails the ISA check, so round through a f32 scratch tile.
        ones_sc = pers.tile([P, 64], f32)
        nc.vector.memset(ones_sc[:], 1.0)
        for h in range(HPC):
            nc.vector.tensor_copy(
                v_sb[:, :, h, 0:64],
                ones_sc[:, None, :].to_broadcast((P, 16, 64)),
            )

        def qk_chunk(jc, q4, tag="S"):
            # one 512-token column chunk of q^T (jc 0,1) or k^T (jc 2,3)
            _MARKS.append((f"qk{jc}q{q4}", nc.get_next_instruction_name()))
            pt = psum.tile([P, 512], f32, tag=tag, bufs=3, name=f"qkp_{jc}_{q4}")
            m0 = q4 * 512
            for kc in range(KC):
                nc.tensor.matmul(
                    pt[:],
                    w_sb[:, kc, WCOL[jc]:WCOL[jc] + P],
                    xT_sb[:, kc, m0:m0 + 512],
                    start=(kc == 0), stop=(kc == KC - 1),
                )
            nc.vector.tensor_copy(qk_sb[:, jc, m0:m0 + 512], pt[:])

        def v_chunk(mc, tag="S"):
            # one 128-token chunk of v for all 4 heads, token-major
            _MARKS.append((f"v{mc}", nc.get_next_instruction_name()))
            pt = psum.tile([P, 256], f32, tag=tag, bufs=3, name=f"vp_{mc}")
            for kc in range(KC):
                nc.tensor.matmul(
                    pt[:],
                    xT_sb[:, kc, mc * P:(mc + 1) * P],
                    w_sb[:, kc, WV0:WV0 + 256],
                    start=(kc == 0), stop=(kc == KC - 1),
                )
            nc.vector.tensor_copy(
                v_sb[:, mc, :, 64:128],
                pt[:].rearrange("p (h e) -> p h e", e=64),
            )

        def out_tile(t, tail=False):
            _MARKS.append((f"out{t}", nc.get_next_instruction_name()))
            po = psum.tile([P, 1024], f32, tag="S", bufs=3, name=f"po_{t}")
            for oc in range(2):
                for dc in range(2):
                    nc.tensor.matmul(
                        po[:, oc * 512:(oc + 1) * 512],
                        zn_sb[:, dc, t * P:(t + 1) * P],
                        wo_sb[:, dc, oc * 512:(oc + 1) * 512],
                        start=(dc == 0), stop=(dc == 1),
                    )
            ot = att.tile([P, 1024], f16, tag="o", bufs=4, name=f"ot_{t}")
            # split the psum->sbuf copy across two engines so the S-tag
            # slot frees in ~0.85us instead of 1.6us of serial Pool time;
            # in the tail the exp stream is done, so ACT does one half
            if tail:
                nc.scalar.copy(ot[:, 512:1024], po[:, 512:1024])
            else:
                nc.vector.tensor_copy(ot[:, 512:1024], po[:, 512:1024])
            nc.gpsimd.tensor_copy(ot[:, 0:512], po[:, 0:512])
            nc.scalar.dma_start(out_ap[:, t], ot[:])

        class Blk:
            """One attention block: single head h over queries
            lq*1024:(lq+1)*1024, all 2048 keys.  One head per block keeps
            the z accumulator at 2 psum banks (1 buf), freeing 6 banks for
            a 3-deep S-tile rotation — the exp stream (1038ns/tile on ACT)
            then always runs ahead of the PE (1708ns+inserts per chunk)."""

            def __init__(self, h, lq):
                self.h, self.lq = h, lq
                self.hp = h // 2
                self.l0 = lq * 1024
                self.zt = psum.tile([P, 1024], f32, tag="z", bufs=1,
                                    name=f"z_{h}_{lq}")
                self.Es = {}

            def S_E(self, mc):
                _MARKS.append((f"SE_h{self.h}l{self.lq}_m{mc}", nc.get_next_instruction_name()))
                hp, l0, r0 = self.hp, self.l0, (self.h % 2) * 64
                S = psum.tile([P, 1024], f32, tag="S", bufs=3)
                for q2 in range(2):
                    nc.tensor.matmul(
                        S[:, q2 * 512:(q2 + 1) * 512],
                        qk_sb[r0:r0 + 64, 2 + hp, mc * P:(mc + 1) * P],
                        qk_sb[r0:r0 + 64, hp,
                              l0 + q2 * 512: l0 + (q2 + 1) * 512],
                        start=True, stop=True,
                    )
                E = att.tile([P, 1024], f32r, tag="E", bufs=8)
                nc.scalar.activation(E[:], S[:], Exp, scale=0.125)
                self.Es[mc] = E

            def AV(self, mc):
                _MARKS.append((f"AV_h{self.h}l{self.lq}_m{mc}", nc.get_next_instruction_name()))
                E = self.Es.pop(mc)
                for q2 in range(2):
                    nc.tensor.matmul(
                        self.zt[:, q2 * 512:(q2 + 1) * 512],
                        v_sb[:, mc, self.h, :],
                        E[:, q2 * 512:(q2 + 1) * 512],
                        start=(mc == 0), stop=(mc == 15),
                    )

            def norm(self):
                # multiply z^T by 1/rowsum (rowsums live on partitions 0:63
                # via the ones columns)
                _MARKS.append((f"norm_h{self.h}l{self.lq}", nc.get_next_instruction_name()))
                for qh in range(2):
                    rz = (self.h % 2) * 64
                    sl = slice(qh * 512, (qh + 1) * 512)
                    rsb = att.tile([P, 512], f32, tag="r", bufs=2)
                    nc.vector.reciprocal_approx_fast(out=rsb[0:64, :],
                                                     in_=self.zt[0:64, sl])
                    nc.vector.tensor_tensor(
                        zn_sb[rz:rz + 64, self.hp,
                              self.l0 + qh * 512:self.l0 + (qh + 1) * 512],
                        self.zt[64:128, sl], rsb[0:64, :], Mult,
                    )

        def emit_attention(sched):
            # sched: list of (Blk, inserts).  The AV stream lags S/E by 2
            # chunks so exp latency never stalls the PE, and the first two
            # S/E chunks of block N+1 are emitted inside block N's tail so
            # the ACT engine stays fed across the z-normalization boundary.
            for bi, (blk, ins) in enumerate(sched):
                nxt = sched[bi + 1][0] if bi + 1 < len(sched) else None
                first = (bi == 0)
                for mc in range(16):
                    if first or mc >= 2:
                        blk.S_E(mc)
                    if ins and mc in ins:
                        for fn in ins[mc]:
                            fn()
                    if mc >= 2:
                        blk.AV(mc - 2)
                    if mc >= 14 and nxt is not None:
                        nxt.S_E(mc - 14)
                blk.AV(14)
                blk.AV(15)
                blk.norm()

        # minimal pre-attention set: q pair0 m0:1024, first key/value
        # chunks (tags alternate for double-buffered chains); everything
        # else is spliced into the attention blocks below
        qk_chunk(0, 0)
        qk_chunk(2, 0)
        v_chunk(0)
        qk_chunk(0, 1)
        v_chunk(1)

        emit_attention([
            # h0 l0: carries the rest of v and the k pair0 tail (all
            # gating chunks for itself and h1 l0)
            (Blk(0, 0), {
                0: [lambda: v_chunk(2), lambda: qk_chunk(2, 1)],
                1: [lambda: v_chunk(3)],
                2: [lambda: v_chunk(4)],
                3: [lambda: v_chunk(5), lambda: qk_chunk(2, 2)],
                4: [lambda: v_chunk(6)],
                5: [lambda: v_chunk(7)],
                6: [lambda: v_chunk(8)],
                7: [lambda: v_chunk(9), lambda: qk_chunk(2, 3)],
                8: [lambda: v_chunk(10)],
                9: [lambda: v_chunk(11)],
                10: [lambda: v_chunk(12)],
                11: [lambda: v_chunk(13)],
                12: [lambda: v_chunk(14)],
                13: [lambda: v_chunk(15)],
            }),
            # h1 l0: k pair1 + q pair1 m0:1024 (gate h2 l0)
            (Blk(1, 0), {
                0: [lambda: qk_chunk(3, 0)],
                2: [lambda: qk_chunk(1, 0)],
                4: [lambda: qk_chunk(3, 1)],
                6: [lambda: qk_chunk(1, 1)],
                8: [lambda: qk_chunk(3, 2)],
                11: [lambda: qk_chunk(3, 3)],
            }),
            # h2 l0: q pair0 m1024:2048 (gates h0 l1)
            (Blk(2, 0), {
                1: [lambda: qk_chunk(0, 2)],
                5: [lambda: qk_chunk(0, 3)],
            }),
            # h3 l0: q pair1 m1024:2048 (gates h2 l1)
            (Blk(3, 0), {
                1: [lambda: qk_chunk(1, 2)],
                5: [lambda: qk_chunk(1, 3)],
            }),
            # l1 blocks carry the first 8 out-proj tiles (tokens 0:1024,
            # whose zn is complete once the four l0 blocks are normalized)
            (Blk(0, 1), {
                1: [lambda: out_tile(0)],
                5: [lambda: out_tile(1)],
                9: [lambda: out_tile(2)],
                13: [lambda: out_tile(3)],
            }),
            (Blk(1, 1), {
                1: [lambda: out_tile(4)],
                6: [lambda: out_tile(5)],
                11: [lambda: out_tile(6)],
            }),
            (Blk(2, 1), {
                1: [lambda: out_tile(7)],
            }),
            (Blk(3, 1), {}),
        ])
        for t in range(8, 16):
            out_tile(t, tail=True)

    nc.compile()
    return nc


def _get_compiled():
    global _COMPILED
    if _COMPILED is None:
        _COMPILED = _build()
    return _COMPILED


def _shard_inputs(x, W_in, W_out):
    in_maps = []
    xTs = [x[:, b, :].T.astype(np.float16) for b in range(B)]
    for c in range(NCORES):
        b = c // 4
        lo = (c % 4) * J
        Wq = W_in[lo:lo + J]
        Wk = W_in[D + lo:D + lo + J]
        Wv = W_in[2 * D + lo:2 * D + lo + J]
        wT = np.concatenate([Wq, Wk, Wv], 0).T.astype(np.float16)
        in_maps.append({
            "xT": xTs[b],
            "wT": np.ascontiguousarray(wT),
            "woT": np.ascontiguousarray(W_out[:, lo:lo + J].T).astype(np.float16),
        })
    return in_maps


def _reference_numpy(q, mask, W_in, b_in, W_out, b_out, num_heads):
    l, b, d = q.shape
    hd = d // num_heads
    qkv = q.reshape(l * b, d) @ W_in.T + b_in
    qkv = qkv.reshape(l, b, 3 * d)
    qh, kh, vh = np.split(qkv, 3, axis=-1)

    def to_heads(t):
        return t.reshape(l, b * num_heads, hd).transpose(1, 0, 2)

    qh, kh, vh = to_heads(qh), to_heads(kh), to_heads(vh)
    qh = qh / np.sqrt(np.float32(hd))
    scores = np.einsum("nld,nmd->nlm", qh, kh) + mask
    scores -= scores.max(axis=-1, keepdims=True)
    e = np.exp(scores)
    attn = e / e.sum(axis=-1, keepdims=True)
    z = np.einsum("nlm,nmd->nld", attn, vh)
    z = z.transpose(1, 0, 2).reshape(l * b, d)
    z = z @ W_out.T + b_out
    return z.reshape(l, b, d).astype(np.float32)


def kernel(q, k, v, mask, W_in, b_in, W_out, b_out, num_heads):
    num_heads = int(num_heads)
    q = np.asarray(q, dtype=np.float32)
    W_in = np.asarray(W_in, dtype=np.float32)
    W_out = np.asarray(W_out, dtype=np.float32)
    b_in = np.asarray(b_in, dtype=np.float32)
    b_out = np.asarray(b_out, dtype=np.float32)
    mask = np.asarray(mask, dtype=np.float32)

    if (
        num_heads != H
        or q.shape != (L, B, D)
        or W_in.shape != (3 * D, D)
        or W_out.shape != (D, D)
        or np.any(mask)
        or np.any(b_in)
    ):
        return _reference_numpy(q, mask, W_in, b_in, W_out, b_out, num_heads)

    from concourse import bass_utils

    nc = _get_compiled()
    in_maps = _shard_inputs(q, W_in, W_out)
    res = bass_utils.run_bass_kernel_spmd(
        nc, in_maps, core_ids=list(range(NCORES))
    )

    out = np.zeros((L, B, D), dtype=np.float32)
    for c in range(NCORES):
        out[:, c // 4, :] += res.results[c]["out_p"].astype(np.float32)
    out += b_out
    return out


# revision 23
# speedup vs baseline: 1.0125x; 1.0125x over previous
"""Multi-head attention layer (L=2048, B=2, D=1024, H=16) on 8 Trainium2 cores.

Sharding: batch*heads across cores — core c handles batch c//4, heads
4*(c%4)..4*(c%4)+4.  Tensor-parallel W_in column slice (per-head) and W_out
row slice; per-core partial outputs are summed on the host (2 groups of 4).

Device program (identical SPMD program, per-core data), organized as a
single PE conveyor:
  - 7 input DMAs: packed weights wT=[Wqk|Wv] (2), xT in m-quarters (4),
    woT (1).  All fp16.  Fewer DMAs = less serialized HWDGE generation
    (625ns each) and the m-quarter order lets attention start as soon as
    the first half of the tokens has landed.
  - projections are emitted as fine chunks (512-token psum tiles for q/k,
    128-token for v) and spliced just-in-time into the attention blocks'
    PE instruction stream wherever the exp pipeline would otherwise be
    the pacer; only the minimal prefix (q pair0 m0:1024, k pair0 m0:1024,
    v mc0..3) runs before attention block 0.
  - attention inner loop is software-pipelined: AV(mc-1) is emitted after
    S(mc), so the exp (ACT engine, ~1?s/tile) never stalls the PE.
  - AV uses interleaved ones-columns in v_sb so one matmul produces both
    z^T and the softmax row sums (cost model charges by output columns,
    so the extra 64 M-rows are free).
  - out-proj psum tiles ride the S-tag rotation; their copies run on the
    (otherwise idle) Pool engine and the output DMAs are issued from the
    ACT queue to keep SP/HWDGE free for input streaming.
"""

import sys

for _p in ("/opt/trn_rl_repo",):
    if _p not in sys.path:
        sys.path.append(_p)

import numpy as np

L, B, D, H = 2048, 2, 1024, 16
HD = 64
NCORES = 8
HPC = 4              # heads per core
J = HPC * HD         # 256 per-core head-dim slice
KC = D // 128        # 8 contraction chunks
P = 128

_COMPILED = None
_MARKS = []


def _build():
    import concourse.bacc as bacc
    import concourse.mybir as mybir
    import concourse.tile as tile
    from contextlib import ExitStack

    f32 = mybir.dt.float32
    f32r = mybir.dt.float32r
    f16 = mybir.dt.float16
    Exp = mybir.ActivationFunctionType.Exp
    Mult = mybir.AluOpType.mult

    del _MARKS[:]
    nc = bacc.Bacc("TRN2", target_bir_lowering=False, debug=False)

    xT_d = nc.dram_tensor("xT", (D, L), f16, kind="ExternalInput")
    w_d = nc.dram_tensor("wT", (D, 3 * J), f16, kind="ExternalInput")
    wo_d = nc.dram_tensor("woT", (J, D), f16, kind="ExternalInput")
    out_d = nc.dram_tensor("out_p", (L, D), f16, kind="ExternalOutput")

    with tile.TileContext(nc) as tc, ExitStack() as ctx:
        pers = ctx.enter_context(tc.tile_pool(name="pers", bufs=1))
        psum = ctx.enter_context(tc.tile_pool(name="psum", bufs=2, space="PSUM"))
        att = ctx.enter_context(tc.tile_pool(name="att", bufs=3))

        qk_sb = pers.tile([P, 4, L], f16)           # chunks 0,1: q^T; 2,3: k^T
        v_sb = pers.tile([P, 16, HPC, P], f32r)     # ones cols 0:64, v 64:128
        zn_sb = pers.tile([P, 2, L], f16)           # normalized z^T
        wo_sb = pers.tile([P, 2, D], f16)
        xT_sb = pers.tile([P, KC, L], f16)
        w_sb = pers.tile([P, KC, 3 * J], f16)       # [qk 0:512 | v 512:768]

        out_ap = out_d.ap().rearrange("(t p) o -> p t o", p=P)
        xT_ap = xT_d.ap().rearrange("(kc p) m -> p kc m", p=P)
        w_ap = w_d.ap().rearrange("(kc p) j -> p kc j", p=P)
        wo_ap = wo_d.ap().rearrange("(dc p) o -> p dc o", p=P)

        # input DMAs: weight chunks interleaved with x quarters so the
        # first projection chains can start as early as possible (chains
        # run kc 0..7, so w kc0:4 + xQ0 unblocks the first 4 matmuls)
        nc.sync.dma_start(w_sb[:, 0:4], w_ap[:, 0:4])
        nc.sync.dma_start(xT_sb[:, :, 0:512], xT_ap[:, :, 0:512])
        nc.sync.dma_start(w_sb[:, 4:8], w_ap[:, 4:8])
        for q4 in range(1, 4):
            sl = slice(q4 * 512, (q4 + 1) * 512)
            nc.sync.dma_start(xT_sb[:, :, sl], xT_ap[:, :, sl])
        nc.sync.dma_start(wo_sb[:], wo_ap[:])

        # ones columns at 0:64 for every head — keeps the softmax sums on
        # psum partitions 0-63 where the custom-DVE reciprocal works (it
        # silently corrupts at base partition 64).  memset on an f32r
        # tile fails the ISA check, so round through a f32 scratch tile.
        ones_sc = pers.tile([P, 64], f32)
        warm_sb = pers.tile([P, 512], f32)
        nc.vector.memset(warm_sb[:], 0.0)
        warm_ps = psum.tile([P, 512], f32, tag="S", bufs=3, name="warm")
        for wi in range(6):
            nc.tensor.matmul(warm_ps[:], warm_sb[:, 0:128], warm_sb[:],
                             start=(wi == 0), stop=(wi == 5))
        nc.vector.memset(ones_sc[:], 1.0)
        for h in range(HPC):
            nc.vector.tensor_copy(
                v_sb[:, :, h, 0:64],
                ones_sc[:, None, :].to_broadcast((P, 16, 64)),
            )

        def qk_chunk(jc, q4, tag="S"):
            # one 512-token column chunk of q^T (jc 0,1) or k^T (jc 2,3)
            _MARKS.append((f"qk{jc}q{q4}", nc.get_next_instruction_name()))
            pt = psum.tile([P, 512], f32, tag=tag, bufs=3, name=f"qkp_{jc}_{q4}")
            m0 = q4 * 512
            for kc in range(KC):
                nc.tensor.matmul(
                    pt[:],
                    w_sb[:, kc, jc * P:(jc + 1) * P],
                    xT_sb[:, kc, m0:m0 + 512],
                    start=(kc == 0), stop=(kc == KC - 1),
                )
            nc.vector.tensor_copy(qk_sb[:, jc, m0:m0 + 512], pt[:])

        def v_chunk(mc, tag="S"):
            # one 128-token chunk of v for all 4 heads, token-major
            _MARKS.append((f"v{mc}", nc.get_next_instruction_name()))
            pt = psum.tile([P, 256], f32, tag=tag, bufs=3, name=f"vp_{mc}")
            for kc in range(KC):
                nc.tensor.matmul(
                    pt[:],
                    xT_sb[:, kc, mc * P:(mc + 1) * P],
                    w_sb[:, kc, 512:768],
                    start=(kc == 0), stop=(kc == KC - 1),
                )
            nc.vector.tensor_copy(
                v_sb[:, mc, :, 64:128],
                pt[:].rearrange("p (h e) -> p h e", e=64),
            )

        def out_tile(t, tail=False):
            _MARKS.append((f"out{t}", nc.get_next_instruction_name()))
            po = psum.tile([P, 1024], f32, tag="S", bufs=3, name=f"po_{t}")
            for oc in range(2):
                for dc in range(2):
                    nc.tensor.matmul(
                        po[:, oc * 512:(oc + 1) * 512],
                        zn_sb[:, dc, t * P:(t + 1) * P],
                        wo_sb[:, dc, oc * 512:(oc + 1) * 512],
                        start=(dc == 0), stop=(dc == 1),
                    )
            ot = att.tile([P, 1024], f16, tag="o", bufs=4, name=f"ot_{t}")
            # split the psum->sbuf copy across two engines so the S-tag
            # slot frees in ~0.85us instead of 1.6us of serial Pool time;
            # in the tail the exp stream is done, so ACT does one half
            if tail:
                nc.scalar.copy(ot[:, 512:1024], po[:, 512:1024])
                nc.gpsimd.tensor_copy(ot[:, 0:512], po[:, 0:512])
                nc.sync.dma_start(out_ap[:, t], ot[:])
            else:
                nc.vector.tensor_copy(ot[:, 512:1024], po[:, 512:1024])
                nc.gpsimd.tensor_copy(ot[:, 0:512], po[:, 0:512])
                nc.scalar.dma_start(out_ap[:, t], ot[:])

        class Blk:
            """One attention block: single head h over queries
            lq*1024:(lq+1)*1024, all 2048 keys.  One head per block keeps
            the z accumulator at 2 psum banks (1 buf), freeing 6 banks for
            a 3-deep S-tile rotation — the exp stream (1038ns/tile on ACT)
            then always runs ahead of the PE (1708ns+inserts per chunk)."""

            def __init__(self, h, lq):
                self.h, self.lq = h, lq
                self.hp = h // 2
                self.l0 = lq * 1024
                self.zt = psum.tile([P, 1024], f32, tag="z", bufs=1,
                                    name=f"z_{h}_{lq}")
                self.Es = {}

            def S_E(self, mc):
                _MARKS.append((f"SE_h{self.h}l{self.lq}_m{mc}", nc.get_next_instruction_name()))
                hp, l0, r0 = self.hp, self.l0, (self.h % 2) * 64
                S = psum.tile([P, 1024], f32, tag="S", bufs=3)
                for q2 in range(2):
                    nc.tensor.matmul(
                        S[:, q2 * 512:(q2 + 1) * 512],
                        qk_sb[r0:r0 + 64, 2 + hp, mc * P:(mc + 1) * P],
                        qk_sb[r0:r0 + 64, hp,
                              l0 + q2 * 512: l0 + (q2 + 1) * 512],
                        start=True, stop=True,
                    )
                E = att.tile([P, 1024], f32r, tag="E", bufs=8)
                nc.scalar.activation(E[:], S[:], Exp, scale=0.125)
                self.Es[mc] = E

            def AV(self, mc):
                _MARKS.append((f"AV_h{self.h}l{self.lq}_m{mc}", nc.get_next_instruction_name()))
                E = self.Es.pop(mc)
                for q2 in range(2):
                    nc.tensor.matmul(
                        self.zt[:, q2 * 512:(q2 + 1) * 512],
                        v_sb[:, mc, self.h, :],
                        E[:, q2 * 512:(q2 + 1) * 512],
                        start=(mc == 0), stop=(mc == 15),
                    )

            def norm(self):
                # multiply z^T by 1/rowsum (rowsums live on partitions 0:63
                # via the ones columns)
                _MARKS.append((f"norm_h{self.h}l{self.lq}", nc.get_next_instruction_name()))
                for qh in range(2):
                    rz = (self.h % 2) * 64
                    sl = slice(qh * 512, (qh + 1) * 512)
                    rsb = att.tile([P, 512], f32, tag="r", bufs=2)
                    nc.vector.reciprocal_approx_fast(out=rsb[0:64, :],
                                                     in_=self.zt[0:64, sl])
                    nc.vector.tensor_tensor(
                        zn_sb[rz:rz + 64, self.hp,
                              self.l0 + qh * 512:self.l0 + (qh + 1) * 512],
                        self.zt[64:128, sl], rsb[0:64, :], Mult,
                    )

        def emit_attention(sched):
            # sched: list of (Blk, inserts).  The AV stream lags S/E by 2
            # chunks so exp latency never stalls the PE, and the first two
            # S/E chunks of block N+1 are emitted inside block N's tail so
            # the ACT engine stays fed across the z-normalization boundary.
            for bi, (blk, ins) in enumerate(sched):
                nxt = sched[bi + 1][0] if bi + 1 < len(sched) else None
                first = (bi == 0)
                for mc in range(16):
                    if first or mc >= 2:
                        blk.S_E(mc)
                    if ins and mc in ins:
                        for fn in ins[mc]:
                            fn()
                    if mc >= 2:
                        blk.AV(mc - 2)
                    if mc >= 14 and nxt is not None:
                        nxt.S_E(mc - 14)
                blk.AV(14)
                blk.AV(15)
                blk.norm()

        # minimal pre-attention set: q pair0 m0:1024, first key/value
        # chunks (tags alternate for double-buffered chains); everything
        # else is spliced into the attention blocks below
        qk_chunk(0, 0)
        qk_chunk(2, 0)
        v_chunk(0)
        qk_chunk(0, 1)
        v_chunk(1)

        emit_attention([
            # h0 l0: carries the rest of v and the k pair0 tail (all
            # gating chunks for itself and h1 l0)
            (Blk(0, 0), {
                0: [lambda: v_chunk(2), lambda: qk_chunk(2, 1)],
                1: [lambda: v_chunk(3)],
                2: [lambda: v_chunk(4)],
                3: [lambda: v_chunk(5), lambda: qk_chunk(2, 2)],
                4: [lambda: v_chunk(6)],
                5: [lambda: v_chunk(7)],
                6: [lambda: v_chunk(8)],
                7: [lambda: v_chunk(9), lambda: qk_chunk(2, 3)],
                8: [lambda: v_chunk(10)],
                9: [lambda: v_chunk(11)],
                10: [lambda: v_chunk(12)],
                11: [lambda: v_chunk(13)],
                12: [lambda: v_chunk(14)],
                13: [lambda: v_chunk(15)],
            }),
            # h1 l0: k pair1 + q pair1 m0:1024 (gate h2 l0)
            (Blk(1, 0), {
                0: [lambda: qk_chunk(3, 0)],
                2: [lambda: qk_chunk(1, 0)],
                4: [lambda: qk_chunk(3, 1)],
                6: [lambda: qk_chunk(1, 1)],
                8: [lambda: qk_chunk(3, 2)],
                11: [lambda: qk_chunk(3, 3)],
            }),
            # h2 l0: q pair0 m1024:2048 (gates h0 l1)
            (Blk(2, 0), {
                1: [lambda: qk_chunk(0, 2)],
                5: [lambda: qk_chunk(0, 3)],
            }),
            # h3 l0: q pair1 m1024:2048 (gates h2 l1)
            (Blk(3, 0), {
                1: [lambda: qk_chunk(1, 2)],
                5: [lambda: qk_chunk(1, 3)],
            }),
            # l1 blocks carry the first 8 out-proj tiles (tokens 0:1024,
            # whose zn is complete once the four l0 blocks are normalized)
            (Blk(0, 1), {
                0: [lambda: out_tile(0)],
                4: [lambda: out_tile(1)],
                8: [lambda: out_tile(2)],
                12: [lambda: out_tile(3)],
            }),
            (Blk(1, 1), {
                0: [lambda: out_tile(4)],
                5: [lambda: out_tile(5)],
                10: [lambda: out_tile(6)],
            }),
            (Blk(2, 1), {
                0: [lambda: out_tile(7)],
            }),
            (Blk(3, 1), {}),
        ])
        for t in range(8, 16):
            out_tile(t, tail=True)

    nc.compile()
    return nc


def _get_compiled():
    global _COMPILED
    if _COMPILED is None:
        _COMPILED = _build()
    return _COMPILED


def _shard_inputs(x, W_in, W_out):
    in_maps = []
    xTs = [x[:, b, :].T.astype(np.float16) for b in range(B)]
    for c in range(NCORES):
        b = c // 4
        lo = (c % 4) * J
        Wq = W_in[lo:lo + J]
        Wk = W_in[D + lo:D + lo + J]
        Wv = W_in[2 * D + lo:2 * D + lo + J]
        wT = np.concatenate([Wq, Wk, Wv], 0).T.astype(np.float16)
        in_maps.append({
            "xT": xTs[b],
            "wT": np.ascontiguousarray(wT),
            "woT": np.ascontiguousarray(W_out[:, lo:lo + J].T).astype(np.float16),
        })
    return in_maps


def _reference_numpy(q, mask, W_in, b_in, W_out, b_out, num_heads):
    l, b, d = q.shape
    hd = d // num_heads
    qkv = q.reshape(l * b, d) @ W_in.T + b_in
    qkv = qkv.reshape(l, b, 3 * d)
    qh, kh, vh = np.split(qkv, 3, axis=-1)

    def to_heads(t):
        return t.reshape(l, b * num_heads, hd).transpose(1, 0, 2)

    qh, kh, vh = to_heads(qh), to_heads(kh), to_heads(vh)
    qh = qh / np.sqrt(np.float32(hd))
    scores = np.einsum("nld,nmd->nlm", qh, kh) + mask
    scores -= scores.max(axis=-1, keepdims=True)
    e = np.exp(scores)
    attn = e / e.sum(axis=-1, keepdims=True)
    z = np.einsum("nlm,nmd->nld", attn, vh)
    z = z.transpose(1, 0, 2).reshape(l * b, d)
    z = z @ W_out.T + b_out
    return z.reshape(l, b, d).astype(np.float32)


def kernel(q, k, v, mask, W_in, b_in, W_out, b_out, num_heads):
    num_heads = int(num_heads)
    q = np.asarray(q, dtype=np.float32)
    W_in = np.asarray(W_in, dtype=np.float32)
    W_out = np.asarray(W_out, dtype=np.float32)
    b_in = np.asarray(b_in, dtype=np.float32)
    b_out = np.asarray(b_out, dtype=np.float32)
    mask = np.asarray(mask, dtype=np.float32)

    if (
        num_heads != H
        or q.shape != (L, B, D)
        or W_in.shape != (3 * D, D)
        or W_out.shape != (D, D)
        or np.any(mask)
        or np.any(b_in)
    ):
        return _reference_numpy(q, mask, W_in, b_in, W_out, b_out, num_heads)

    from concourse import bass_utils

    nc = _get_compiled()
    in_maps = _shard_inputs(q, W_in, W_out)
    res = bass_utils.run_bass_kernel_spmd(
        nc, in_maps, core_ids=list(range(NCORES))
    )

    out = np.zeros((L, B, D), dtype=np.float32)
    for c in range(NCORES):
        out[:, c // 4, :] += res.results[c]["out_p"].astype(np.float32)
    out += b_out
    return out


# revision 24
# speedup vs baseline: 1.0481x; 1.0351x over previous
"""Multi-head attention layer (L=2048, B=2, D=1024, H=16) on 8 Trainium2 cores.

Sharding: batch*heads across cores — core c handles batch c//4, heads
4*(c%4)..4*(c%4)+4.  Tensor-parallel W_in column slice (per-head) and W_out
row slice; per-core partial outputs are summed on the host (2 groups of 4).

Device program (identical SPMD program, per-core data), organized as a
single PE conveyor:
  - 7 input DMAs: packed weights wT=[Wqk|Wv] (2), xT in m-quarters (4),
    woT (1).  All fp16.  Fewer DMAs = less serialized HWDGE generation
    (625ns each) and the m-quarter order lets attention start as soon as
    the first half of the tokens has landed.
  - projections are emitted as fine chunks (512-token psum tiles for q/k,
    128-token for v) and spliced just-in-time into the attention blocks'
    PE instruction stream wherever the exp pipeline would otherwise be
    the pacer; only the minimal prefix (q pair0 m0:1024, k pair0 m0:1024,
    v mc0..3) runs before attention block 0.
  - attention inner loop is software-pipelined: AV(mc-1) is emitted after
    S(mc), so the exp (ACT engine, ~1?s/tile) never stalls the PE.
  - AV uses interleaved ones-columns in v_sb so one matmul produces both
    z^T and the softmax row sums (cost model charges by output columns,
    so the extra 64 M-rows are free).
  - out-proj psum tiles ride the S-tag rotation; their copies run on the
    (otherwise idle) Pool engine and the output DMAs are issued from the
    ACT queue to keep SP/HWDGE free for input streaming.
"""

import sys

for _p in ("/opt/trn_rl_repo",):
    if _p not in sys.path:
        sys.path.append(_p)

import numpy as np

L, B, D, H = 2048, 2, 1024, 16
HD = 64
NCORES = 8
HPC = 4              # heads per core
J = HPC * HD         # 256 per-core head-dim slice
KC = D // 128        # 8 contraction chunks
P = 128

_COMPILED = None
_MARKS = []


def _build():
    import concourse.bacc as bacc
    import concourse.mybir as mybir
    import concourse.tile as tile
    from contextlib import ExitStack

    f32 = mybir.dt.float32
    f32r = mybir.dt.float32r
    f16 = mybir.dt.float16
    Exp = mybir.ActivationFunctionType.Exp
    Mult = mybir.AluOpType.mult

    del _MARKS[:]
    nc = bacc.Bacc("TRN2", target_bir_lowering=False, debug=False)

    xT_d = nc.dram_tensor("xT", (D, L), f16, kind="ExternalInput")
    w_d = nc.dram_tensor("wT", (D, 3 * J), f16, kind="ExternalInput")
    wo_d = nc.dram_tensor("woT", (J, D), f16, kind="ExternalInput")
    out_d = nc.dram_tensor("out_p", (L, D), f16, kind="ExternalOutput")

    with tile.TileContext(nc) as tc, ExitStack() as ctx:
        pers = ctx.enter_context(tc.tile_pool(name="pers", bufs=1))
        psum = ctx.enter_context(tc.tile_pool(name="psum", bufs=2, space="PSUM"))
        att = ctx.enter_context(tc.tile_pool(name="att", bufs=3))

        qk_sb = pers.tile([P, 4, L], f16)           # chunks 0,1: q^T; 2,3: k^T
        v_sb = pers.tile([P, 16, HPC, P], f32r)     # ones cols 0:64, v 64:128
        zn_sb = pers.tile([P, 2, L], f16)           # normalized z^T
        wo_sb = pers.tile([P, 2, D], f16)
        xT_sb = pers.tile([P, KC, L], f16)
        w_sb = pers.tile([P, KC, 3 * J], f16)       # [qk 0:512 | v 512:768]

        out_ap = out_d.ap().rearrange("(t p) o -> p t o", p=P)
        xT_ap = xT_d.ap().rearrange("(kc p) m -> p kc m", p=P)
        w_ap = w_d.ap().rearrange("(kc p) j -> p kc j", p=P)
        wo_ap = wo_d.ap().rearrange("(dc p) o -> p dc o", p=P)

        # input DMAs: weight chunks interleaved with x quarters so the
        # first projection chains can start as early as possible (chains
        # run kc 0..7, so w kc0:4 + xQ0 unblocks the first 4 matmuls)
        nc.sync.dma_start(w_sb[:, 0:4], w_ap[:, 0:4])
        nc.sync.dma_start(xT_sb[:, :, 0:512], xT_ap[:, :, 0:512])
        nc.sync.dma_start(w_sb[:, 4:8], w_ap[:, 4:8])
        for q4 in range(1, 4):
            sl = slice(q4 * 512, (q4 + 1) * 512)
            nc.sync.dma_start(xT_sb[:, :, sl], xT_ap[:, :, sl])
        nc.sync.dma_start(wo_sb[:], wo_ap[:])

        # ones columns at 0:64 for every head — keeps the softmax sums on
        # psum partitions 0-63 where the custom-DVE reciprocal works (it
        # silently corrupts at base partition 64).  memset on an f32r
        # tile fails the ISA check, so round through a f32 scratch tile.
        ones_sc = pers.tile([P, 64], f32)
        warm_sb = pers.tile([P, 512], f16)
        nc.vector.memset(warm_sb[:], 0.0)
        warm_ps = psum.tile([P, 512], f32, tag="S", bufs=3, name="warm")
        for wi in range(20):
            nc.tensor.matmul(warm_ps[:], warm_sb[:, 0:128], warm_sb[:],
                             start=(wi == 0), stop=(wi == 19))
        nc.vector.memset(ones_sc[:], 1.0)
        for h in range(HPC):
            nc.vector.tensor_copy(
                v_sb[:, :, h, 0:64],
                ones_sc[:, None, :].to_broadcast((P, 16, 64)),
            )

        def qk_chunk(jc, q4, tag="S"):
            # one 512-token column chunk of q^T (jc 0,1) or k^T (jc 2,3)
            _MARKS.append((f"qk{jc}q{q4}", nc.get_next_instruction_name()))
            pt = psum.tile([P, 512], f32, tag=tag, bufs=3, name=f"qkp_{jc}_{q4}")
            m0 = q4 * 512
            for kc in range(KC):
                nc.tensor.matmul(
                    pt[:],
                    w_sb[:, kc, jc * P:(jc + 1) * P],
                    xT_sb[:, kc, m0:m0 + 512],
                    start=(kc == 0), stop=(kc == KC - 1),
                )
            nc.vector.tensor_copy(qk_sb[:, jc, m0:m0 + 512], pt[:])

        def v_chunk(mc, tag="S"):
            # one 128-token chunk of v for all 4 heads, token-major
            _MARKS.append((f"v{mc}", nc.get_next_instruction_name()))
            pt = psum.tile([P, 256], f32, tag=tag, bufs=3, name=f"vp_{mc}")
            for kc in range(KC):
                nc.tensor.matmul(
                    pt[:],
                    xT_sb[:, kc, mc * P:(mc + 1) * P],
                    w_sb[:, kc, 512:768],
                    start=(kc == 0), stop=(kc == KC - 1),
                )
            nc.vector.tensor_copy(
                v_sb[:, mc, :, 64:128],
                pt[:].rearrange("p (h e) -> p h e", e=64),
            )

        def out_tile(t, tail=False):
            _MARKS.append((f"out{t}", nc.get_next_instruction_name()))
            po = psum.tile([P, 1024], f32, tag="S", bufs=3, name=f"po_{t}")
            for oc in range(2):
                for dc in range(2):
                    nc.tensor.matmul(
                        po[:, oc * 512:(oc + 1) * 512],
                        zn_sb[:, dc, t * P:(t + 1) * P],
                        wo_sb[:, dc, oc * 512:(oc + 1) * 512],
                        start=(dc == 0), stop=(dc == 1),
                    )
            ot = att.tile([P, 1024], f16, tag="o", bufs=4, name=f"ot_{t}")
            # split the psum->sbuf copy across two engines so the S-tag
            # slot frees in ~0.85us instead of 1.6us of serial Pool time;
            # in the tail the exp stream is done, so ACT does one half
            if tail:
                nc.scalar.copy(ot[:, 512:1024], po[:, 512:1024])
                nc.gpsimd.tensor_copy(ot[:, 0:512], po[:, 0:512])
                nc.sync.dma_start(out_ap[:, t], ot[:])
            else:
                nc.gpsimd.tensor_copy(ot[:, 512:1024], po[:, 512:1024])
                nc.gpsimd.tensor_copy(ot[:, 0:512], po[:, 0:512])
                nc.scalar.dma_start(out_ap[:, t], ot[:])

        class Blk:
            """One attention block: single head h over queries
            lq*1024:(lq+1)*1024, all 2048 keys.  One head per block keeps
            the z accumulator at 2 psum banks (1 buf), freeing 6 banks for
            a 3-deep S-tile rotation — the exp stream (1038ns/tile on ACT)
            then always runs ahead of the PE (1708ns+inserts per chunk)."""

            def __init__(self, h, lq):
                self.h, self.lq = h, lq
                self.hp = h // 2
                self.l0 = lq * 1024
                self.zt = psum.tile([P, 1024], f32, tag="z", bufs=1,
                                    name=f"z_{h}_{lq}")
                self.Es = {}

            def S_E(self, mc):
                _MARKS.append((f"SE_h{self.h}l{self.lq}_m{mc}", nc.get_next_instruction_name()))
                hp, l0, r0 = self.hp, self.l0, (self.h % 2) * 64
                S = psum.tile([P, 1024], f32, tag="S", bufs=3)
                for q2 in range(2):
                    nc.tensor.matmul(
                        S[:, q2 * 512:(q2 + 1) * 512],
                        qk_sb[r0:r0 + 64, 2 + hp, mc * P:(mc + 1) * P],
                        qk_sb[r0:r0 + 64, hp,
                              l0 + q2 * 512: l0 + (q2 + 1) * 512],
                        start=True, stop=True,
                    )
                E = att.tile([P, 1024], f32r, tag="E", bufs=8)
                nc.scalar.activation(E[:], S[:], Exp, scale=0.125)
                self.Es[mc] = E

            def AV(self, mc):
                _MARKS.append((f"AV_h{self.h}l{self.lq}_m{mc}", nc.get_next_instruction_name()))
                E = self.Es.pop(mc)
                for q2 in range(2):
                    nc.tensor.matmul(
                        self.zt[:, q2 * 512:(q2 + 1) * 512],
                        v_sb[:, mc, self.h, :],
                        E[:, q2 * 512:(q2 + 1) * 512],
                        start=(mc == 0), stop=(mc == 15),
                    )

            def zcopy(self):
                # single DVE copy parks z in SBUF so the psum accumulator
                # frees fast; the normalization reads the parked copy later
                _MARKS.append((f"zcopy_h{self.h}l{self.lq}", nc.get_next_instruction_name()))
                self.zc = att.tile([P, 1024], f32, tag="zc", bufs=2,
                                   name=f"zc_{self.h}_{self.lq}")
                nc.vector.tensor_copy(self.zc[:], self.zt[:])

            def norm(self, src_=None):
                # multiply z^T by 1/rowsum (rowsums live on partitions 0:63
                # via the ones columns)
                _MARKS.append((f"norm_h{self.h}l{self.lq}", nc.get_next_instruction_name()))
                zt = self.zt if src_ is None else src_
                for qh in range(2):
                    rz = (self.h % 2) * 64
                    sl = slice(qh * 512, (qh + 1) * 512)
                    rsb = att.tile([P, 512], f32, tag="r", bufs=2)
                    nc.vector.reciprocal_approx_fast(out=rsb[0:64, :],
                                                     in_=zt[0:64, sl])
                    nc.vector.tensor_tensor(
                        zn_sb[rz:rz + 64, self.hp,
                              self.l0 + qh * 512:self.l0 + (qh + 1) * 512],
                        zt[64:128, sl], rsb[0:64, :], Mult,
                    )

            def finish_norm(self):
                self.norm(src_=self.zc)

        def emit_attention(sched):
            # sched: list of (Blk, inserts).  The AV stream lags S/E by 2
            # chunks so exp latency never stalls the PE, and the first two
            # S/E chunks of block N+1 are emitted inside block N's tail so
            # the ACT engine stays fed across the z-normalization boundary.
            prev = None
            for bi, (blk, ins) in enumerate(sched):
                nxt = sched[bi + 1][0] if bi + 1 < len(sched) else None
                first = (bi == 0)
                for mc in range(16):
                    if first or mc >= 2:
                        blk.S_E(mc)
                    if mc == 1 and prev is not None:
                        prev.finish_norm()
                    if ins and mc in ins:
                        for fn in ins[mc]:
                            fn()
                    if mc >= 2:
                        blk.AV(mc - 2)
                    if mc >= 14 and nxt is not None:
                        nxt.S_E(mc - 14)
                blk.AV(14)
                blk.AV(15)
                if nxt is not None:
                    blk.zcopy()
                    prev = blk
                else:
                    blk.norm()

        # minimal pre-attention set: q pair0 m0:1024, first key/value
        # chunks (tags alternate for double-buffered chains); everything
        # else is spliced into the attention blocks below
        qk_chunk(0, 0)
        qk_chunk(2, 0)
        v_chunk(0)
        qk_chunk(0, 1)
        v_chunk(1)

        emit_attention([
            # h0 l0: carries the rest of v and the k pair0 tail (all
            # gating chunks for itself and h1 l0)
            (Blk(0, 0), {
                0: [lambda: v_chunk(2), lambda: qk_chunk(2, 1)],
                1: [lambda: v_chunk(3)],
                2: [lambda: v_chunk(4)],
                3: [lambda: v_chunk(5), lambda: qk_chunk(2, 2)],
                4: [lambda: v_chunk(6)],
                5: [lambda: v_chunk(7)],
                6: [lambda: v_chunk(8)],
                7: [lambda: v_chunk(9), lambda: qk_chunk(2, 3)],
                8: [lambda: v_chunk(10)],
                9: [lambda: v_chunk(11)],
                10: [lambda: v_chunk(12)],
                11: [lambda: v_chunk(13)],
                12: [lambda: v_chunk(14)],
                13: [lambda: v_chunk(15)],
            }),
            # h1 l0: k pair1 + q pair1 m0:1024 (gate h2 l0)
            (Blk(1, 0), {
                0: [lambda: qk_chunk(3, 0)],
                2: [lambda: qk_chunk(1, 0)],
                4: [lambda: qk_chunk(3, 1)],
                6: [lambda: qk_chunk(1, 1)],
                8: [lambda: qk_chunk(3, 2)],
                11: [lambda: qk_chunk(3, 3)],
            }),
            # h2 l0: q pair0 m1024:2048 (gates h0 l1)
            (Blk(2, 0), {
                1: [lambda: qk_chunk(0, 2)],
                5: [lambda: qk_chunk(0, 3)],
            }),
            # h3 l0: q pair1 m1024:2048 (gates h2 l1)
            (Blk(3, 0), {
                1: [lambda: qk_chunk(1, 2)],
                5: [lambda: qk_chunk(1, 3)],
            }),
            # l1 blocks carry the first 8 out-proj tiles (tokens 0:1024,
            # whose zn is complete once the four l0 blocks are normalized)
            (Blk(0, 1), {
                0: [lambda: out_tile(0)],
                4: [lambda: out_tile(1)],
                8: [lambda: out_tile(2)],
                12: [lambda: out_tile(3)],
            }),
            (Blk(1, 1), {
                0: [lambda: out_tile(4)],
                5: [lambda: out_tile(5)],
                10: [lambda: out_tile(6)],
            }),
            (Blk(2, 1), {
                0: [lambda: out_tile(7)],
            }),
            (Blk(3, 1), {}),
        ])
        for t in range(8, 16):
            out_tile(t, tail=True)

    nc.compile()
    return nc


def _get_compiled():
    global _COMPILED
    if _COMPILED is None:
        _COMPILED = _build()
    return _COMPILED


def _shard_inputs(x, W_in, W_out):
    in_maps = []
    xTs = [x[:, b, :].T.astype(np.float16) for b in range(B)]
    for c in range(NCORES):
        b = c // 4
        lo = (c % 4) * J
        Wq = W_in[lo:lo + J]
        Wk = W_in[D + lo:D + lo + J]
        Wv = W_in[2 * D + lo:2 * D + lo + J]
        wT = np.concatenate([Wq, Wk, Wv], 0).T.astype(np.float16)
        in_maps.append({
            "xT": xTs[b],
            "wT": np.ascontiguousarray(wT),
            "woT": np.ascontiguousarray(W_out[:, lo:lo + J].T).astype(np.float16),
        })
    return in_maps


def _reference_numpy(q, mask, W_in, b_in, W_out, b_out, num_heads):
    l, b, d = q.shape
    hd = d // num_heads
    qkv = q.reshape(l * b, d) @ W_in.T + b_in
    qkv = qkv.reshape(l, b, 3 * d)
    qh, kh, vh = np.split(qkv, 3, axis=-1)

    def to_heads(t):
        return t.reshape(l, b * num_heads, hd).transpose(1, 0, 2)

    qh, kh, vh = to_heads(qh), to_heads(kh), to_heads(vh)
    qh = qh / np.sqrt(np.float32(hd))
    scores = np.einsum("nld,nmd->nlm", qh, kh) + mask
    scores -= scores.max(axis=-1, keepdims=True)
    e = np.exp(scores)
    attn = e / e.sum(axis=-1, keepdims=True)
    z = np.einsum("nlm,nmd->nld", attn, vh)
    z = z.transpose(1, 0, 2).reshape(l * b, d)
    z = z @ W_out.T + b_out
    return z.reshape(l, b, d).astype(np.float32)


def kernel(q, k, v, mask, W_in, b_in, W_out, b_out, num_heads):
    num_heads = int(num_heads)
    q = np.asarray(q, dtype=np.float32)
    W_in = np.asarray(W_in, dtype=np.float32)
    W_out = np.asarray(W_out, dtype=np.float32)
    b_in = np.asarray(b_in, dtype=np.float32)
    b_out = np.asarray(b_out, dtype=np.float32)
    mask = np.asarray(mask, dtype=np.float32)

    if (
        num_heads != H
        or q.shape != (L, B, D)
        or W_in.shape != (3 * D, D)
        or W_out.shape != (D, D)
        or np.any(mask)
        or np.any(b_in)
    ):
        return _reference_numpy(q, mask, W_in, b_in, W_out, b_out, num_heads)

    from concourse import bass_utils

    nc = _get_compiled()
    in_maps = _shard_inputs(q, W_in, W_out)
    res = bass_utils.run_bass_kernel_spmd(
        nc, in_maps, core_ids=list(range(NCORES))
    )

    out = np.zeros((L, B, D), dtype=np.float32)
    for c in range(NCORES):
        out[:, c // 4, :] += res.results[c]["out_p"].astype(np.float32)
    out += b_out
    return out
